# revision 17
# baseline (speedup 1.0000x reference)
"""HardTripletLoss on 8 Trainium2 NeuronCores (Bass/Tile).

Math
----
reference: emb = l2_normalize(embeddings); dist = cdist(emb, emb);
  pos_stat[i] = mean_{j: same class, j!=i} dist[i,j]
  neg_stat[i] = min_{j: diff class} dist[i,j]
  loss = mean over valid rows of relu(pos_stat - neg_stat + 1)

For unit vectors dist^2 = 2 - 2*ghat where ghat = N @ N.T.  We fold the
class mask into the GEMM itself: with Y = onehot(labels) [64, B],

  A = [ N.T ; -Y ]  (rhs side; the lhsT +2*Y block ships separately)

and contracting lhsT = [N.T ; +sqrt(2)*Y] (the +Y block from a separate
small per-core tile) against rhs = [N.T ; -sqrt(2)*Y] gives
P = ghat - 2*S  (S = same-class indicator incl diagonal).  Then per row:
  masked positive dists = sqrt(2*relu(-1 - P))   (diff-class & diagonal -> 0)
  hardest negative      = sqrt(relu(2 - 2*rowmax(P)))
      (rowmax(P) = max over diff-class ghat, since same-class P <= -1+eps)

Sharding: rows split 512/core (data parallel).  Every core holds all 4096
columns of A in SBUF (10.5 MB) as 8 slabs of 512 columns; slab order is
rotated per core so each core's first-loaded slab contains its own shard
columns (the matmul stationary operand), letting the GEMM start after the
first 1.3 MB DMA.  Row stats are order-invariant (sum/max over columns).

Host does only input marshaling (normalize+transpose+onehot packing,
O(B*D), 0.02% of the FLOPs) and the final scalar mean over 4096
device-computed per-row losses.
"""

import sys

if "/opt/trn_rl_repo" not in sys.path:
    sys.path.insert(0, "/opt/trn_rl_repo")

import ml_dtypes
import numpy as np


import concourse.bass as bass
import concourse.bacc as bacc
import concourse.mybir as mybir
import concourse.tile as tile
from concourse.bass_utils import run_bass_kernel_spmd

F32 = mybir.dt.float32
F32R = mybir.dt.float32r
BF16 = mybir.dt.bfloat16
GEMM_DT = BF16  # bf16: fast weight load + half DMA; f32r fallback if accuracy demands
ALU = mybir.AluOpType
ACTF = mybir.ActivationFunctionType
AXX = mybir.AxisListType.X

B = 4096
D = 512
C = 64
NCORES = 8
SHARD = B // NCORES          # 512 rows per core
MT = SHARD // 128            # 4 m-tiles per core
NJ = 8                       # column slabs of 512
KC = 5                       # k-chunks of 128 (4 data + 1 +/- onehot)
SLABW = KC * 512             # 2560
# psum chunk structure: slabs 0 and 1 get single-slab chunks (they arrive
# first and pace the pipeline start); later slabs are paired for bigger,
# cheaper epilogue ops.  One chunk = (slab set, m-tile) -> one PSUM tile.
JSETS = [[0], [1], [2, 3], [4, 5], [6, 7]]
MT_ = 4
CHUNKS = [(js, m) for js in JSETS for m in range(MT_)]
NCHUNK = len(CHUNKS)         # 20

MARGIN = 1.0


def _build_nc():
    nc = bacc.Bacc(
        "TRN2",
        target_bir_lowering=False,
        debug=False,
        enable_asserts=False,
        num_devices=NCORES,
    )
    atp = nc.dram_tensor("atp", [NJ, 128, SLABW], GEMM_DT, kind="ExternalInput")
    yl = nc.dram_tensor("yl", [C, SHARD], GEMM_DT, kind="ExternalInput")
    NCOL = NCHUNK + 1  # +1: last chunk's epilogue runs as two halves
    stats_d = nc.dram_tensor("stats", [128, 2 * NCOL], F32, kind="ExternalOutput")

    with tile.TileContext(nc) as tc:
        with (
            tc.tile_pool(name="slabs", bufs=1) as slabs,
            tc.tile_pool(name="psum", bufs=4, space=bass.MemorySpace.PSUM) as psum,
            tc.tile_pool(name="scr", bufs=4) as scr,
            tc.tile_pool(name="stat", bufs=1) as stat,
        ):
            # small lhsT-side one-hot block first: every group's c=4 matmul
            # needs it, so it must not queue behind 10 MB of slab DMA
            ylt = stat.tile([C, SHARD], GEMM_DT, name="ylt", tag="ylt")
            nc.sync.dma_start(ylt[:], yl.ap())
            # slab 0 (the stationary-operand columns) split into per-k-chunk
            # pieces so the first matmuls start after ~130 KB, not 650 KB
            s0c = []
            for c in range(KC):
                t = slabs.tile([128, 512], GEMM_DT, name=f"s0c{c}", tag=f"s0c{c}")
                nc.sync.dma_start(t[:], atp[0, :, c * 512 : (c + 1) * 512])
                s0c.append(t)
            slab_t = [None]
            for j in range(1, NJ):
                t = slabs.tile([128, SLABW], GEMM_DT, name=f"slab{j}", tag=f"slab{j}")
                nc.sync.dma_start(t[:], atp[j])
                slab_t.append(t)

            # per-(m, group) partial stats in one tile: cols [0, NCOL) pos
            # row-sums, cols [NCOL, 2*NCOL) row-maxes
            parts = stat.tile([128, 2 * NCOL], F32, name="parts", tag="parts")

            # bias constants for ACT (float biases need pre-registered const
            # APs, so build [128,1] tiles explicitly)
            bias_c = {}
            for bname, bval in [("m1", -1.0), ("m2", -2.0), ("z", 0.0)]:
                bt = stat.tile([128, 1], F32, name=f"bc_{bname}", tag=f"bc_{bname}")
                nc.gpsimd.memset(bt[:], bval)
                bias_c[bname] = bt

            warm = stat.tile([128, 1], F32, name="warm", tag="warm")
            nc.scalar.activation(warm[:], bias_c["z"][:], ACTF.Relu)
            nc.scalar.activation(warm[:], warm[:], ACTF.Sqrt, bias=bias_c["z"][:])

            # PE warm-up: ~4us of dummy matmuls while the first slab DMA is in
            # flight, so the HAM clock-gate opens before the real GEMM starts
            warm_w = stat.tile([128, 128], GEMM_DT, name="warm_w", tag="warm_w")
            warm_x = stat.tile([128, 512], GEMM_DT, name="warm_x", tag="warm_x")
            nc.gpsimd.memset(warm_w[:], 0.0)
            nc.gpsimd.memset(warm_x[:], 0.0)
            wpt = psum.tile([128, 512], F32, name="wpt", tag="pt")
            for _ in range(9):
                nc.tensor.matmul(wpt[:], warm_w[:], warm_x[:], start=True, stop=True)

            for k, (jset, m) in enumerate(CHUNKS):
                w = len(jset) * 512
                pt = psum.tile([128, w], F32, name="pt", tag="pt")
                # K=64 one-hot chunk first: its LDW can't pull ahead as well,
                # so pay that at the chunk boundary where a hiccup exists anyway
                for ci, c in enumerate([KC - 1] + list(range(KC - 1))):
                    for jj, j in enumerate(jset):
                        if c < KC - 1:
                            lhsT = s0c[c][:, m * 128 : (m + 1) * 128]
                            rhs = (
                                s0c[c][:, :]
                                if j == 0
                                else slab_t[j][:, c * 512 : (c + 1) * 512]
                            )
                        else:
                            lhsT = ylt[:, m * 128 : (m + 1) * 128]
                            rhs = (
                                s0c[c][0:C, :]
                                if j == 0
                                else slab_t[j][0:C, c * 512 : (c + 1) * 512]
                            )
                        nc.tensor.matmul(
                            pt[:, jj * 512 : (jj + 1) * 512],
                            lhsT,
                            rhs,
                            start=(ci == 0),
                            stop=(ci == KC - 1),
                        )
                last = k == NCHUNK - 1
                t1 = scr.tile([128, 1024], F32, name="t1", tag="t1")
                d1 = scr.tile([128, 1024], F32, name="d1", tag="d1")
                if last:
                    # two halves, clamp on ACT and DVE in parallel, to
                    # shorten the end-of-kernel serial chain
                    h = w // 2
                    nc.scalar.activation(
                        t1[:, :h], pt[:, :h], ACTF.Relu,
                        bias=bias_c["m1"][:], scale=-1.0,
                    )
                    nc.vector.tensor_scalar(
                        t1[:, h:w], pt[:, h:], -1.0, None, op0=ALU.min
                    )
                    nc.scalar.activation(
                        d1[:, :h], t1[:, :h], ACTF.Sqrt,
                        bias=bias_c["z"][:], scale=2.0,
                        accum_out=parts[:, k : k + 1],
                    )
                    nc.scalar.activation(
                        d1[:, h:w], t1[:, h:w], ACTF.Sqrt,
                        bias=bias_c["m2"][:], scale=-2.0,
                        accum_out=parts[:, k + 1 : k + 2],
                    )
                    nc.vector.tensor_reduce(
                        parts[:, NCOL + k : NCOL + k + 1],
                        pt[:, :h], axis=AXX, op=ALU.max,
                    )
                    nc.vector.tensor_reduce(
                        parts[:, NCOL + k + 1 : NCOL + k + 2],
                        pt[:, h:], axis=AXX, op=ALU.max,
                    )
                else:
                    if k % 2 == 0:
                        # ACT: t1 = relu(-P - 1); d = sqrt(2*t1)
                        nc.scalar.activation(
                            t1[:, :w], pt[:], ACTF.Relu,
                            bias=bias_c["m1"][:], scale=-1.0,
                        )
                        nc.scalar.activation(
                            d1[:, :w], t1[:, :w], ACTF.Sqrt,
                            bias=bias_c["z"][:], scale=2.0,
                            accum_out=parts[:, k : k + 1],
                        )
                    else:
                        # DVE: t1 = min(P, -1); d = sqrt(-2*t1 - 2)
                        nc.vector.tensor_scalar(
                            t1[:, :w], pt[:], -1.0, None, op0=ALU.min
                        )
                        nc.scalar.activation(
                            d1[:, :w], t1[:, :w], ACTF.Sqrt,
                            bias=bias_c["m2"][:], scale=-2.0,
                            accum_out=parts[:, k : k + 1],
                        )
                    nc.vector.tensor_reduce(
                        parts[:, NCOL + k : NCOL + k + 1],
                        pt[:], axis=AXX, op=ALU.max,
                    )

            nc.sync.dma_start(stats_d.ap(), parts[:])

    nc.compile()
    return nc


_NC_CACHE: dict = {}


def _get_nc():
    if "nc" not in _NC_CACHE:
        _NC_CACHE["nc"] = _build_nc()
    return _NC_CACHE["nc"]


def _prep_inputs(embeddings: np.ndarray, labels: np.ndarray):
    E = np.asarray(embeddings, dtype=np.float32)
    L = np.asarray(labels).astype(np.int64)
    assert E.shape == (B, D) and L.shape == (B,)

    nrm = np.maximum(np.linalg.norm(E.astype(np.float32), axis=1), 1e-12)
    N = (E / nrm[:, None].astype(np.float32)).astype(np.float32)

    Y = (L[None, :] == np.arange(C, dtype=np.int64)[:, None]).astype(np.float32)
    # chunk 4 partitions 0:64 hold -Y (the rhs side); the +2*Y lhsT side
    # ships separately per core (yl).  Partitions 64:128 stay zero.
    AT = np.zeros((KC * 128, B), dtype=np.float32)
    AT[:D] = N.T
    AT[D : D + C] = -Y

    # slabs[j][p, c*512+x] = AT[128c+p, 512j+x]
    slabs8 = np.ascontiguousarray(
        AT.reshape(KC, 128, NJ, 512)
        .transpose(2, 1, 0, 3)
        .reshape(NJ, 128, SLABW)
        .astype(ml_dtypes.bfloat16)
    )

    cnt = np.bincount(L, minlength=C)
    pos_cnt = cnt[L] - 1
    neg_cnt = B - cnt[L]
    invc = (1.0 / np.maximum(pos_cnt, 1)).astype(np.float32)
    valid = ((pos_cnt > 0) & (neg_cnt > 0)).astype(np.float32)

    in_maps = []
    for r in range(NCORES):
        rows = slice(SHARD * r, SHARD * (r + 1))
        in_maps.append(
            {
                "atp": np.ascontiguousarray(np.roll(slabs8, -r, axis=0)),
                "yl": np.ascontiguousarray((2.0 * Y[:, rows]).astype(ml_dtypes.bfloat16)),
            }
        )
    return in_maps, (invc, valid)


def _finish(results, aux):
    invc, valid = aux
    NCOL = NCHUNK + 1
    pos_sum = np.empty(B, dtype=np.float32)
    max_p = np.empty(B, dtype=np.float32)
    for r in range(NCORES):
        st = np.asarray(results[r]["stats"])
        pp, mp = st[:, :NCOL], st[:, NCOL:]
        psum_m = np.zeros((128, MT), dtype=np.float32)
        pmax_m = np.full((128, MT), -np.inf, dtype=np.float32)
        for k, (jset, m) in enumerate(CHUNKS):
            psum_m[:, m] += pp[:, k]
            pmax_m[:, m] = np.maximum(pmax_m[:, m], mp[:, k])
        # split last chunk's second half lives in the extra column
        m_last = CHUNKS[-1][1]
        psum_m[:, m_last] += pp[:, NCHUNK]
        pmax_m[:, m_last] = np.maximum(pmax_m[:, m_last], mp[:, NCHUNK])
        rows = slice(SHARD * r, SHARD * (r + 1))
        pos_sum[rows] = psum_m.T.reshape(SHARD)
        max_p[rows] = pmax_m.T.reshape(SHARD)
    pos_stat = pos_sum * invc
    neg_stat = np.sqrt(np.maximum(2.0 - 2.0 * max_p, 0.0), dtype=np.float32)
    per_row = np.maximum(pos_stat - neg_stat + MARGIN, 0.0) * valid
    n_valid = float(valid.sum())
    total = float(per_row.sum(dtype=np.float32))
    out = total / max(n_valid, 1.0) if n_valid > 0 else 0.0
    return np.array(out, dtype=np.float32)


def kernel(embeddings, labels, _run_kwargs=None):
    nc = _get_nc()
    in_maps, aux = _prep_inputs(embeddings, labels)
    res = run_bass_kernel_spmd(
        nc, in_maps, core_ids=list(range(NCORES)), **(_run_kwargs or {})
    )
    out = _finish(res.results, aux)
    if _run_kwargs:
        return out, res
    return out


# revision 18
# speedup vs baseline: 1.0050x; 1.0050x over previous
"""HardTripletLoss on 8 Trainium2 NeuronCores (Bass/Tile).

Math
----
reference: emb = l2_normalize(embeddings); dist = cdist(emb, emb);
  pos_stat[i] = mean_{j: same class, j!=i} dist[i,j]
  neg_stat[i] = min_{j: diff class} dist[i,j]
  loss = mean over valid rows of relu(pos_stat - neg_stat + 1)

For unit vectors dist^2 = 2 - 2*ghat where ghat = N @ N.T.  We fold the
class mask into the GEMM itself: with Y = onehot(labels) [64, B],

  A = [ N.T ; -Y ]  (rhs side; the lhsT +2*Y block ships separately)

and contracting lhsT = [N.T ; +sqrt(2)*Y] (the +Y block from a separate
small per-core tile) against rhs = [N.T ; -sqrt(2)*Y] gives
P = ghat - 2*S  (S = same-class indicator incl diagonal).  Then per row:
  masked positive dists = sqrt(2*relu(-1 - P))   (diff-class & diagonal -> 0)
  hardest negative      = sqrt(relu(2 - 2*rowmax(P)))
      (rowmax(P) = max over diff-class ghat, since same-class P <= -1+eps)

Sharding: rows split 512/core (data parallel).  Every core holds all 4096
columns of A in SBUF (10.5 MB) as 8 slabs of 512 columns; slab order is
rotated per core so each core's first-loaded slab contains its own shard
columns (the matmul stationary operand), letting the GEMM start after the
first 1.3 MB DMA.  Row stats are order-invariant (sum/max over columns).

Host does only input marshaling (normalize+transpose+onehot packing,
O(B*D), 0.02% of the FLOPs) and the final scalar mean over 4096
device-computed per-row losses.
"""

import sys

if "/opt/trn_rl_repo" not in sys.path:
    sys.path.insert(0, "/opt/trn_rl_repo")

import ml_dtypes
import numpy as np


import concourse.bass as bass
import concourse.bacc as bacc
import concourse.mybir as mybir
import concourse.tile as tile
from concourse.bass_utils import run_bass_kernel_spmd

F32 = mybir.dt.float32
F32R = mybir.dt.float32r
BF16 = mybir.dt.bfloat16
GEMM_DT = BF16  # bf16: fast weight load + half DMA; f32r fallback if accuracy demands
ALU = mybir.AluOpType
ACTF = mybir.ActivationFunctionType
AXX = mybir.AxisListType.X

B = 4096
D = 512
C = 64
NCORES = 8
SHARD = B // NCORES          # 512 rows per core
MT = SHARD // 128            # 4 m-tiles per core
NJ = 8                       # column slabs of 512
KC = 5                       # k-chunks of 128 (4 data + 1 +/- onehot)
SLABW = KC * 512             # 2560
# psum chunk structure: slabs 0 and 1 get single-slab chunks (they arrive
# first and pace the pipeline start); later slabs are paired for bigger,
# cheaper epilogue ops.  One chunk = (slab set, m-tile) -> one PSUM tile.
JSETS = [[0], [1], [2, 3], [4, 5], [6, 7]]
MT_ = 4
CHUNKS = [(js, m) for js in JSETS for m in range(MT_)]
NCHUNK = len(CHUNKS)         # 20

MARGIN = 1.0


def _build_nc():
    nc = bacc.Bacc(
        "TRN2",
        target_bir_lowering=False,
        debug=False,
        enable_asserts=False,
        num_devices=NCORES,
    )
    atp = nc.dram_tensor("atp", [NJ, 128, SLABW], GEMM_DT, kind="ExternalInput")
    yl = nc.dram_tensor("yl", [C, SHARD], GEMM_DT, kind="ExternalInput")
    NCOL = NCHUNK + 1  # +1: last chunk's epilogue runs as two halves
    stats_d = nc.dram_tensor("stats", [128, 2 * NCOL], F32, kind="ExternalOutput")

    with tile.TileContext(nc) as tc:
        with (
            tc.tile_pool(name="slabs", bufs=1) as slabs,
            tc.tile_pool(name="psum", bufs=4, space=bass.MemorySpace.PSUM) as psum,
            tc.tile_pool(name="scr", bufs=4) as scr,
            tc.tile_pool(name="stat", bufs=1) as stat,
        ):
            # small lhsT-side one-hot block first: every group's c=4 matmul
            # needs it, so it must not queue behind 10 MB of slab DMA
            ylt = stat.tile([C, SHARD], GEMM_DT, name="ylt", tag="ylt")
            nc.sync.dma_start(ylt[:], yl.ap())
            # slab 0 (the stationary-operand columns) split into per-k-chunk
            # pieces so the first matmuls start after ~130 KB, not 650 KB
            s0c = []
            for c in range(KC):
                t = slabs.tile([128, 512], GEMM_DT, name=f"s0c{c}", tag=f"s0c{c}")
                nc.sync.dma_start(t[:], atp[0, :, c * 512 : (c + 1) * 512])
                s0c.append(t)
            slab_t = [None]
            for j in range(1, NJ):
                t = slabs.tile([128, SLABW], GEMM_DT, name=f"slab{j}", tag=f"slab{j}")
                nc.sync.dma_start(t[:], atp[j])
                slab_t.append(t)

            # per-(m, group) partial stats in one tile: cols [0, NCOL) pos
            # row-sums, cols [NCOL, 2*NCOL) row-maxes
            parts = stat.tile([128, 2 * NCOL], F32, name="parts", tag="parts")

            # bias constants for ACT (float biases need pre-registered const
            # APs, so build [128,1] tiles explicitly)
            bias_c = {}
            for bname, bval in [("m1", -1.0), ("m2", -2.0), ("z", 0.0)]:
                bt = stat.tile([128, 1], F32, name=f"bc_{bname}", tag=f"bc_{bname}")
                nc.gpsimd.memset(bt[:], bval)
                bias_c[bname] = bt

            warm = stat.tile([128, 1], F32, name="warm", tag="warm")
            nc.scalar.activation(warm[:], bias_c["z"][:], ACTF.Relu)
            nc.scalar.activation(warm[:], warm[:], ACTF.Sqrt, bias=bias_c["z"][:])

            # PE warm-up: ~4us of dummy matmuls while the first slab DMA is in
            # flight, so the HAM clock-gate opens before the real GEMM starts
            warm_w = stat.tile([128, 128], GEMM_DT, name="warm_w", tag="warm_w")
            warm_x = stat.tile([128, 512], GEMM_DT, name="warm_x", tag="warm_x")
            nc.gpsimd.memset(warm_w[:], 0.0)
            nc.gpsimd.memset(warm_x[:], 0.0)
            wpt = psum.tile([128, 512], F32, name="wpt", tag="pt")
            for _ in range(9):
                nc.tensor.matmul(wpt[:], warm_w[:], warm_x[:], start=True, stop=True)

            for k, (jset, m) in enumerate(CHUNKS):
                w = len(jset) * 512
                pt = psum.tile([128, w], F32, name="pt", tag="pt")
                for ci, c in enumerate(range(KC)):
                    for jj, j in enumerate(jset):
                        if c < KC - 1:
                            lhsT = s0c[c][:, m * 128 : (m + 1) * 128]
                            rhs = (
                                s0c[c][:, :]
                                if j == 0
                                else slab_t[j][:, c * 512 : (c + 1) * 512]
                            )
                        else:
                            lhsT = ylt[:, m * 128 : (m + 1) * 128]
                            rhs = (
                                s0c[c][0:C, :]
                                if j == 0
                                else slab_t[j][0:C, c * 512 : (c + 1) * 512]
                            )
                        nc.tensor.matmul(
                            pt[:, jj * 512 : (jj + 1) * 512],
                            lhsT,
                            rhs,
                            start=(ci == 0),
                            stop=(ci == KC - 1),
                        )
                last = k == NCHUNK - 1
                t1 = scr.tile([128, 1024], F32, name="t1", tag="t1")
                d1 = scr.tile([128, 1024], F32, name="d1", tag="d1")
                if last:
                    # two halves, clamp on ACT and DVE in parallel, to
                    # shorten the end-of-kernel serial chain
                    h = w // 2
                    nc.scalar.activation(
                        t1[:, :h], pt[:, :h], ACTF.Relu,
                        bias=bias_c["m1"][:], scale=-1.0,
                    )
                    nc.vector.tensor_scalar(
                        t1[:, h:w], pt[:, h:], -1.0, None, op0=ALU.min
                    )
                    nc.scalar.activation(
                        d1[:, :h], t1[:, :h], ACTF.Sqrt,
                        bias=bias_c["z"][:], scale=2.0,
                        accum_out=parts[:, k : k + 1],
                    )
                    nc.scalar.activation(
                        d1[:, h:w], t1[:, h:w], ACTF.Sqrt,
                        bias=bias_c["m2"][:], scale=-2.0,
                        accum_out=parts[:, k + 1 : k + 2],
                    )
                    nc.vector.tensor_reduce(
                        parts[:, NCOL + k : NCOL + k + 1],
                        pt[:, :h], axis=AXX, op=ALU.max,
                    )
                    nc.vector.tensor_reduce(
                        parts[:, NCOL + k + 1 : NCOL + k + 2],
                        pt[:, h:], axis=AXX, op=ALU.max,
                    )
                else:
                    if k % 2 == 0:
                        # ACT: t1 = relu(-P - 1); d = sqrt(2*t1)
                        nc.scalar.activation(
                            t1[:, :w], pt[:], ACTF.Relu,
                            bias=bias_c["m1"][:], scale=-1.0,
                        )
                        nc.scalar.activation(
                            d1[:, :w], t1[:, :w], ACTF.Sqrt,
                            bias=bias_c["z"][:], scale=2.0,
                            accum_out=parts[:, k : k + 1],
                        )
                    else:
                        # DVE: t1 = min(P, -1); d = sqrt(-2*t1 - 2)
                        nc.vector.tensor_scalar(
                            t1[:, :w], pt[:], -1.0, None, op0=ALU.min
                        )
                        nc.scalar.activation(
                            d1[:, :w], t1[:, :w], ACTF.Sqrt,
                            bias=bias_c["m2"][:], scale=-2.0,
                            accum_out=parts[:, k : k + 1],
                        )
                    nc.vector.tensor_reduce(
                        parts[:, NCOL + k : NCOL + k + 1],
                        pt[:], axis=AXX, op=ALU.max,
                    )

            nc.sync.dma_start(stats_d.ap(), parts[:])

    nc.compile()
    return nc


_NC_CACHE: dict = {}


def _get_nc():
    if "nc" not in _NC_CACHE:
        _NC_CACHE["nc"] = _build_nc()
    return _NC_CACHE["nc"]


def _prep_inputs(embeddings: np.ndarray, labels: np.ndarray):
    E = np.asarray(embeddings, dtype=np.float32)
    L = np.asarray(labels).astype(np.int64)
    assert E.shape == (B, D) and L.shape == (B,)

    nrm = np.maximum(np.linalg.norm(E.astype(np.float32), axis=1), 1e-12)
    N = (E / nrm[:, None].astype(np.float32)).astype(np.float32)

    Y = (L[None, :] == np.arange(C, dtype=np.int64)[:, None]).astype(np.float32)
    # chunk 4 partitions 0:64 hold -Y (the rhs side); the +2*Y lhsT side
    # ships separately per core (yl).  Partitions 64:128 stay zero.
    AT = np.zeros((KC * 128, B), dtype=np.float32)
    AT[:D] = N.T
    AT[D : D + C] = -Y

    # slabs[j][p, c*512+x] = AT[128c+p, 512j+x]
    slabs8 = np.ascontiguousarray(
        AT.reshape(KC, 128, NJ, 512)
        .transpose(2, 1, 0, 3)
        .reshape(NJ, 128, SLABW)
        .astype(ml_dtypes.bfloat16)
    )

    cnt = np.bincount(L, minlength=C)
    pos_cnt = cnt[L] - 1
    neg_cnt = B - cnt[L]
    invc = (1.0 / np.maximum(pos_cnt, 1)).astype(np.float32)
    valid = ((pos_cnt > 0) & (neg_cnt > 0)).astype(np.float32)

    in_maps = []
    for r in range(NCORES):
        rows = slice(SHARD * r, SHARD * (r + 1))
        in_maps.append(
            {
                "atp": np.ascontiguousarray(np.roll(slabs8, -r, axis=0)),
                "yl": np.ascontiguousarray((2.0 * Y[:, rows]).astype(ml_dtypes.bfloat16)),
            }
        )
    return in_maps, (invc, valid)


def _finish(results, aux):
    invc, valid = aux
    NCOL = NCHUNK + 1
    pos_sum = np.empty(B, dtype=np.float32)
    max_p = np.empty(B, dtype=np.float32)
    for r in range(NCORES):
        st = np.asarray(results[r]["stats"])
        pp, mp = st[:, :NCOL], st[:, NCOL:]
        psum_m = np.zeros((128, MT), dtype=np.float32)
        pmax_m = np.full((128, MT), -np.inf, dtype=np.float32)
        for k, (jset, m) in enumerate(CHUNKS):
            psum_m[:, m] += pp[:, k]
            pmax_m[:, m] = np.maximum(pmax_m[:, m], mp[:, k])
        # split last chunk's second half lives in the extra column
        m_last = CHUNKS[-1][1]
        psum_m[:, m_last] += pp[:, NCHUNK]
        pmax_m[:, m_last] = np.maximum(pmax_m[:, m_last], mp[:, NCHUNK])
        rows = slice(SHARD * r, SHARD * (r + 1))
        pos_sum[rows] = psum_m.T.reshape(SHARD)
        max_p[rows] = pmax_m.T.reshape(SHARD)
    pos_stat = pos_sum * invc
    neg_stat = np.sqrt(np.maximum(2.0 - 2.0 * max_p, 0.0), dtype=np.float32)
    per_row = np.maximum(pos_stat - neg_stat + MARGIN, 0.0) * valid
    n_valid = float(valid.sum())
    total = float(per_row.sum(dtype=np.float32))
    out = total / max(n_valid, 1.0) if n_valid > 0 else 0.0
    return np.array(out, dtype=np.float32)


def kernel(embeddings, labels, _run_kwargs=None):
    nc = _get_nc()
    in_maps, aux = _prep_inputs(embeddings, labels)
    res = run_bass_kernel_spmd(
        nc, in_maps, core_ids=list(range(NCORES)), **(_run_kwargs or {})
    )
    out = _finish(res.results, aux)
    if _run_kwargs:
        return out, res
    return out


# revision 19
# speedup vs baseline: 1.0579x; 1.0527x over previous
"""HardTripletLoss on 8 Trainium2 NeuronCores (Bass/Tile).

Math
----
reference: emb = l2_normalize(embeddings); dist = cdist(emb, emb);
  pos_stat[i] = mean_{j: same class, j!=i} dist[i,j]
  neg_stat[i] = min_{j: diff class} dist[i,j]
  loss = mean over valid rows of relu(pos_stat - neg_stat + 1)

For unit vectors dist^2 = 2 - 2*ghat where ghat = N @ N.T.  We fold the
class mask into the GEMM itself: with Y = onehot(labels) [64, B],

  A = [ N.T ; -Y ]  (rhs side; the lhsT +2*Y block ships separately)

and contracting lhsT = [N.T ; +sqrt(2)*Y] (the +Y block from a separate
small per-core tile) against rhs = [N.T ; -sqrt(2)*Y] gives
P = ghat - 2*S  (S = same-class indicator incl diagonal).  Then per row:
  masked positive dists = sqrt(2*relu(-1 - P))   (diff-class & diagonal -> 0)
  hardest negative      = sqrt(relu(2 - 2*rowmax(P)))
      (rowmax(P) = max over diff-class ghat, since same-class P <= -1+eps)

Sharding: rows split 512/core (data parallel).  Every core holds all 4096
columns of A in SBUF (10.5 MB) as 8 slabs of 512 columns; slab order is
rotated per core so each core's first-loaded slab contains its own shard
columns (the matmul stationary operand), letting the GEMM start after the
first 1.3 MB DMA.  Row stats are order-invariant (sum/max over columns).

Host does only input marshaling (normalize+transpose+onehot packing,
O(B*D), 0.02% of the FLOPs) and the final scalar mean over 4096
device-computed per-row losses.
"""

import sys

if "/opt/trn_rl_repo" not in sys.path:
    sys.path.insert(0, "/opt/trn_rl_repo")

import ml_dtypes
import numpy as np


import concourse.bass as bass
import concourse.bacc as bacc
import concourse.mybir as mybir
import concourse.tile as tile
from concourse.bass_utils import run_bass_kernel_spmd

F32 = mybir.dt.float32
F32R = mybir.dt.float32r
BF16 = mybir.dt.bfloat16
GEMM_DT = BF16  # bf16: fast weight load + half DMA; f32r fallback if accuracy demands
ALU = mybir.AluOpType
ACTF = mybir.ActivationFunctionType
AXX = mybir.AxisListType.X

B = 4096
D = 512
C = 64
NCORES = 8
SHARD = B // NCORES          # 512 rows per core
MT = SHARD // 128            # 4 m-tiles per core
NJ = 8                       # column slabs of 512
KC = 5                       # k-chunks of 128 (4 data + 1 +/- onehot)
SLABW = KC * 512             # 2560
# psum chunk structure: slabs 0 and 1 get single-slab chunks (they arrive
# first and pace the pipeline start); later slabs are paired for bigger,
# cheaper epilogue ops.  One chunk = (slab set, m-tile) -> one PSUM tile.
JSETS = [[0], [1], [2, 3], [4, 5], [6, 7]]
MT_ = 4
CHUNKS = [(js, m) for js in JSETS for m in range(MT_)]
NCHUNK = len(CHUNKS)         # 20

MARGIN = 1.0


def _build_nc():
    nc = bacc.Bacc(
        "TRN2",
        target_bir_lowering=False,
        debug=False,
        enable_asserts=False,
        num_devices=NCORES,
    )
    atp = nc.dram_tensor("atp", [NJ, 128, SLABW], GEMM_DT, kind="ExternalInput")
    yl = nc.dram_tensor("yl", [C, SHARD], GEMM_DT, kind="ExternalInput")
    NCOL = NCHUNK + 1  # +1: last chunk's epilogue runs as two halves
    stats_d = nc.dram_tensor("stats", [128, 2 * NCOL], F32, kind="ExternalOutput")

    with tile.TileContext(nc) as tc:
        with (
            tc.tile_pool(name="slabs", bufs=1) as slabs,
            tc.tile_pool(name="psum", bufs=4, space=bass.MemorySpace.PSUM) as psum,
            tc.tile_pool(name="scr", bufs=3) as scr,
            tc.tile_pool(name="stat", bufs=1) as stat,
        ):
            # small lhsT-side one-hot block first: every group's c=4 matmul
            # needs it, so it must not queue behind 10 MB of slab DMA
            ylt = stat.tile([C, SHARD], GEMM_DT, name="ylt", tag="ylt")
            nc.sync.dma_start(ylt[:], yl.ap())
            # slab 0 (the stationary-operand columns) split into per-k-chunk
            # pieces so the first matmuls start after ~130 KB, not 650 KB
            s0c = []
            for c in range(KC):
                t = slabs.tile([128, 512], GEMM_DT, name=f"s0c{c}", tag=f"s0c{c}")
                nc.sync.dma_start(t[:], atp[0, :, c * 512 : (c + 1) * 512])
                s0c.append(t)
            slab_t = [None]
            for j in range(1, NJ):
                t = slabs.tile([128, SLABW], GEMM_DT, name=f"slab{j}", tag=f"slab{j}")
                nc.sync.dma_start(t[:], atp[j])
                slab_t.append(t)

            # per-(m, group) partial stats in one tile: cols [0, NCOL) pos
            # row-sums, cols [NCOL, 2*NCOL) row-maxes
            parts = stat.tile([128, 2 * NCOL], F32, name="parts", tag="parts")

            # bias constants for ACT (float biases need pre-registered const
            # APs, so build [128,1] tiles explicitly)
            bias_c = {}
            for bname, bval in [("m1", -1.0), ("m2", -2.0), ("z", 0.0)]:
                bt = stat.tile([128, 1], F32, name=f"bc_{bname}", tag=f"bc_{bname}")
                nc.gpsimd.memset(bt[:], bval)
                bias_c[bname] = bt

            warm = stat.tile([128, 1], F32, name="warm", tag="warm")
            nc.scalar.activation(warm[:], bias_c["z"][:], ACTF.Relu)
            nc.scalar.activation(warm[:], warm[:], ACTF.Sqrt, bias=bias_c["z"][:])

            # PE warm-up: ~4us of dummy matmuls while the first slab DMA is in
            # flight, so the HAM clock-gate opens before the real GEMM starts
            warm_w = stat.tile([128, 128], GEMM_DT, name="warm_w", tag="warm_w")
            warm_x = stat.tile([128, 512], GEMM_DT, name="warm_x", tag="warm_x")
            nc.gpsimd.memset(warm_w[:], 0.0)
            nc.gpsimd.memset(warm_x[:], 0.0)
            wpt = psum.tile([128, 512], F32, name="wpt", tag="pt")
            for _ in range(9):
                nc.tensor.matmul(wpt[:], warm_w[:], warm_x[:], start=True, stop=True)

            for k, (jset, m) in enumerate(CHUNKS):
                w = len(jset) * 512
                pt = psum.tile([128, w], F32, name="pt", tag="pt")
                for ci, c in enumerate(range(KC)):
                    for jj, j in enumerate(jset):
                        if c < KC - 1:
                            lhsT = s0c[c][:, m * 128 : (m + 1) * 128]
                            rhs = (
                                s0c[c][:, :]
                                if j == 0
                                else slab_t[j][:, c * 512 : (c + 1) * 512]
                            )
                        else:
                            lhsT = ylt[:, m * 128 : (m + 1) * 128]
                            rhs = (
                                s0c[c][0:C, :]
                                if j == 0
                                else slab_t[j][0:C, c * 512 : (c + 1) * 512]
                            )
                        nc.tensor.matmul(
                            pt[:, jj * 512 : (jj + 1) * 512],
                            lhsT,
                            rhs,
                            start=(ci == 0),
                            stop=(ci == KC - 1),
                        )
                last = k == NCHUNK - 1
                t1 = scr.tile([128, 1024], F32, name="t1", tag="t1")
                d1 = scr.tile([128, 1024], F32, name="d1", tag="d1")
                if last:
                    # two halves, clamp on ACT and DVE in parallel, to
                    # shorten the end-of-kernel serial chain
                    h = w // 2
                    nc.scalar.activation(
                        t1[:, :h], pt[:, :h], ACTF.Relu,
                        bias=bias_c["m1"][:], scale=-1.0,
                    )
                    nc.vector.tensor_scalar(
                        t1[:, h:w], pt[:, h:], -1.0, None, op0=ALU.min
                    )
                    nc.scalar.activation(
                        d1[:, :h], t1[:, :h], ACTF.Sqrt,
                        bias=bias_c["z"][:], scale=2.0,
                        accum_out=parts[:, k : k + 1],
                    )
                    nc.scalar.activation(
                        d1[:, h:w], t1[:, h:w], ACTF.Sqrt,
                        bias=bias_c["m2"][:], scale=-2.0,
                        accum_out=parts[:, k + 1 : k + 2],
                    )
                    nc.vector.tensor_reduce(
                        parts[:, NCOL + k : NCOL + k + 1],
                        pt[:, :h], axis=AXX, op=ALU.max,
                    )
                    nc.vector.tensor_reduce(
                        parts[:, NCOL + k + 1 : NCOL + k + 2],
                        pt[:, h:], axis=AXX, op=ALU.max,
                    )
                else:
                    if k % 2 == 0:
                        # ACT: t1 = relu(-P - 1); d = sqrt(2*t1)
                        nc.scalar.activation(
                            t1[:, :w], pt[:], ACTF.Relu,
                            bias=bias_c["m1"][:], scale=-1.0,
                        )
                        nc.scalar.activation(
                            d1[:, :w], t1[:, :w], ACTF.Sqrt,
                            bias=bias_c["z"][:], scale=2.0,
                            accum_out=parts[:, k : k + 1],
                        )
                    else:
                        # DVE: t1 = min(P, -1); d = sqrt(-2*t1 - 2)
                        nc.vector.tensor_scalar(
                            t1[:, :w], pt[:], -1.0, None, op0=ALU.min
                        )
                        nc.scalar.activation(
                            d1[:, :w], t1[:, :w], ACTF.Sqrt,
                            bias=bias_c["m2"][:], scale=-2.0,
                            accum_out=parts[:, k : k + 1],
                        )
                    nc.vector.tensor_reduce(
                        parts[:, NCOL + k : NCOL + k + 1],
                        pt[:], axis=AXX, op=ALU.max,
                    )

            nc.sync.dma_start(stats_d.ap(), parts[:])

    nc.compile()
    return nc


_NC_CACHE: dict = {}


def _get_nc():
    if "nc" not in _NC_CACHE:
        _NC_CACHE["nc"] = _build_nc()
    return _NC_CACHE["nc"]


def _prep_inputs(embeddings: np.ndarray, labels: np.ndarray):
    E = np.asarray(embeddings, dtype=np.float32)
    L = np.asarray(labels).astype(np.int64)
    assert E.shape == (B, D) and L.shape == (B,)

    nrm = np.maximum(np.linalg.norm(E.astype(np.float32), axis=1), 1e-12)
    N = (E / nrm[:, None].astype(np.float32)).astype(np.float32)

    Y = (L[None, :] == np.arange(C, dtype=np.int64)[:, None]).astype(np.float32)
    # chunk 4 partitions 0:64 hold -Y (the rhs side); the +2*Y lhsT side
    # ships separately per core (yl).  Partitions 64:128 stay zero.
    AT = np.zeros((KC * 128, B), dtype=np.float32)
    AT[:D] = N.T
    AT[D : D + C] = -Y

    # slabs[j][p, c*512+x] = AT[128c+p, 512j+x]
    slabs8 = np.ascontiguousarray(
        AT.reshape(KC, 128, NJ, 512)
        .transpose(2, 1, 0, 3)
        .reshape(NJ, 128, SLABW)
        .astype(ml_dtypes.bfloat16)
    )

    cnt = np.bincount(L, minlength=C)
    pos_cnt = cnt[L] - 1
    neg_cnt = B - cnt[L]
    invc = (1.0 / np.maximum(pos_cnt, 1)).astype(np.float32)
    valid = ((pos_cnt > 0) & (neg_cnt > 0)).astype(np.float32)

    in_maps = []
    for r in range(NCORES):
        rows = slice(SHARD * r, SHARD * (r + 1))
        in_maps.append(
            {
                "atp": np.ascontiguousarray(np.roll(slabs8, -r, axis=0)),
                "yl": np.ascontiguousarray((2.0 * Y[:, rows]).astype(ml_dtypes.bfloat16)),
            }
        )
    return in_maps, (invc, valid)


def _finish(results, aux):
    invc, valid = aux
    NCOL = NCHUNK + 1
    pos_sum = np.empty(B, dtype=np.float32)
    max_p = np.empty(B, dtype=np.float32)
    for r in range(NCORES):
        st = np.asarray(results[r]["stats"])
        pp, mp = st[:, :NCOL], st[:, NCOL:]
        psum_m = np.zeros((128, MT), dtype=np.float32)
        pmax_m = np.full((128, MT), -np.inf, dtype=np.float32)
        for k, (jset, m) in enumerate(CHUNKS):
            psum_m[:, m] += pp[:, k]
            pmax_m[:, m] = np.maximum(pmax_m[:, m], mp[:, k])
        # split last chunk's second half lives in the extra column
        m_last = CHUNKS[-1][1]
        psum_m[:, m_last] += pp[:, NCHUNK]
        pmax_m[:, m_last] = np.maximum(pmax_m[:, m_last], mp[:, NCHUNK])
        rows = slice(SHARD * r, SHARD * (r + 1))
        pos_sum[rows] = psum_m.T.reshape(SHARD)
        max_p[rows] = pmax_m.T.reshape(SHARD)
    pos_stat = pos_sum * invc
    neg_stat = np.sqrt(np.maximum(2.0 - 2.0 * max_p, 0.0), dtype=np.float32)
    per_row = np.maximum(pos_stat - neg_stat + MARGIN, 0.0) * valid
    n_valid = float(valid.sum())
    total = float(per_row.sum(dtype=np.float32))
    out = total / max(n_valid, 1.0) if n_valid > 0 else 0.0
    return np.array(out, dtype=np.float32)


def kernel(embeddings, labels, _run_kwargs=None):
    nc = _get_nc()
    in_maps, aux = _prep_inputs(embeddings, labels)
    res = run_bass_kernel_spmd(
        nc, in_maps, core_ids=list(range(NCORES)), **(_run_kwargs or {})
    )
    out = _finish(res.results, aux)
    if _run_kwargs:
        return out, res
    return out
